# revision 1
# baseline (speedup 1.0000x reference)
"""Neighbor aggregation (GNN message passing) on 8 Trainium2 cores.

out[b, u] = sum_{edges (u, v) in batch b} w_e * H[b, v]    (messages are 16x16 blocks)

Sharding: core (b, h) handles batch b's edges whose destination falls in
dst-half h (h=0: nodes [0, 25088), h=1: [25088, 50048)).  Outputs are disjoint
-> no cross-core reduction.  Within a core, edges are bucketed by 128-node dst
block and by src half (so gather indices fit in int16), padded to a uniform
number of 128-edge groups per bucket.  Device loop per dst block:
  dma_gather 128-row groups of H   (SWDGE bulk gather)
  one-hot weight matrix W[e, d] = w_e * (d == dstloc_e)   (one DVE op)
  PSUM += W.T @ Hgather                                    (fp32 matmul)
  copy PSUM -> SBUF -> DRAM out rows.
"""

import sys

sys.path.insert(0, "/opt/trn_rl_repo")

import numpy as np

import concourse.bacc as bacc
import concourse.tile as tile
from concourse import mybir
from concourse.bass_utils import run_bass_kernel_spmd

B = 4
N_NODES = 50000
HS = 16
C = HS * HS          # 256 floats per message row
P = 128
E = 800000
NBLK = 196           # dst blocks per core (196*128 = 25088 rows of output)
HALF0 = NBLK * P     # dst boundary between the two cores of a batch
SRC_SPLIT = 25000    # src half boundary; local indices stay < 32768 (int16)
NSEG = NBLK * 2      # (block, src-half) buckets per core
N_CORES = 8

_prog_cache: dict[int, object] = {}
_last_in_maps: list | None = None


def _build_program(gh: int):
    """Bass program for all 8 cores; gh = 128-edge groups per (block, src-half)."""
    ngrp = NSEG * gh             # one-hot groups per core
    idx_cols = NSEG * gh * 8     # int16 idx columns (16 idx per column)

    nc = bacc.Bacc("TRN2", target_bir_lowering=False, debug=False)
    h_d = nc.dram_tensor("h", (N_NODES, C), mybir.dt.float32, kind="ExternalInput")
    idx_d = nc.dram_tensor("idx", (P, idx_cols), mybir.dt.int16, kind="ExternalInput")
    mdst_d = nc.dram_tensor("mdst", (P, ngrp), mybir.dt.float32, kind="ExternalInput")
    mw_d = nc.dram_tensor("mw", (P, ngrp), mybir.dt.float32, kind="ExternalInput")
    iota_d = nc.dram_tensor("iota", (P, P), mybir.dt.float32, kind="ExternalInput")
    out_d = nc.dram_tensor("out", (NBLK * P, C), mybir.dt.float32, kind="ExternalOutput")

    h_ap = h_d.ap()
    src_half_aps = (h_ap[0:SRC_SPLIT, :], h_ap[SRC_SPLIT:N_NODES, :])

    with tile.TileContext(nc) as tc:
        with tc.tile_pool(name="const", bufs=1) as cpool, \
             tc.tile_pool(name="gat", bufs=8) as gpool, \
             tc.tile_pool(name="wtile", bufs=4) as wpool, \
             tc.tile_pool(name="otile", bufs=4) as opool, \
             tc.tile_pool(name="psum", bufs=8, space="PSUM") as ppool:
            iota_t = cpool.tile([P, P], mybir.dt.float32)
            nc.sync.dma_start(out=iota_t[:], in_=iota_d.ap())
            mdst_t = cpool.tile([P, ngrp], mybir.dt.float32)
            nc.sync.dma_start(out=mdst_t[:], in_=mdst_d.ap())
            mw_t = cpool.tile([P, ngrp], mybir.dt.float32)
            nc.sync.dma_start(out=mw_t[:], in_=mw_d.ap())
            idx_t = cpool.tile([P, idx_cols], mybir.dt.int16)
            nc.sync.dma_start(out=idx_t[:], in_=idx_d.ap())

            for j in range(NBLK):
                gt = []
                for s in range(2):
                    seg = j * 2 + s
                    # dma_gather tops out at 1024 indices per call; separate
                    # tiles per chunk so the calls don't serialize on WAW
                    parts = []
                    for ci, c0 in enumerate(range(0, gh, 8)):
                        c1 = min(c0 + 8, gh)
                        g = gpool.tile([P, c1 - c0, C], mybir.dt.float32,
                                       tag=f"gat{ci}")
                        nc.gpsimd.dma_gather(
                            out_ap=g[:],
                            in_ap=src_half_aps[s],
                            idxs_ap=idx_t[:, seg * gh * 8 + c0 * 8:seg * gh * 8 + c1 * 8],
                            num_idxs=(c1 - c0) * P,
                            num_idxs_reg=(c1 - c0) * P,
                            elem_size=C,
                        )
                        parts.append((g, c0, c1))
                    gt.append(parts)

                acc = ppool.tile([P, C], mybir.dt.float32, space="PSUM")
                for gi in range(2 * gh):
                    s, gg = divmod(gi, gh)
                    col = (j * 2 + s) * gh + gg
                    g, c0, _ = next(p for p in gt[s] if p[1] <= gg < p[2])
                    W = wpool.tile([P, P], mybir.dt.float32, tag="W")
                    nc.vector.tensor_scalar(
                        out=W[:],
                        in0=iota_t[:],
                        scalar1=mdst_t[:, col:col + 1],
                        scalar2=mw_t[:, col:col + 1],
                        op0=mybir.AluOpType.is_equal,
                        op1=mybir.AluOpType.mult,
                    )
                    nc.tensor.matmul(
                        out=acc[:],
                        lhsT=W[:],
                        rhs=g[:, gg - c0, :],
                        start=(gi == 0),
                        stop=(gi == 2 * gh - 1),
                    )
                ot = opool.tile([P, C], mybir.dt.float32, tag="out")
                nc.any.tensor_copy(out=ot[:], in_=acc[:])
                nc.sync.dma_start(out=out_d.ap()[j * P:(j + 1) * P, :], in_=ot[:])

    nc.compile()
    return nc


def kernel(H, edge_index, edge_weight, node_idx):
    H = np.ascontiguousarray(np.asarray(H), dtype=np.float32)
    edge_index = np.asarray(edge_index)
    edge_weight = np.ascontiguousarray(np.asarray(edge_weight), dtype=np.float32)
    node_idx = np.asarray(node_idx)

    inv = np.argsort(node_idx).astype(np.int64)  # id -> row (identity for arange)
    iota = np.tile(np.arange(P, dtype=np.float32), (P, 1))

    # ---- host bucketing: (core, dst-block, src-half) ----
    per_core = []   # (sloc_sorted, dloc_sorted, w_sorted, counts) per core
    gh = 1
    for b in range(B):
        dst = inv[edge_index[b, :, 0]]
        src = inv[edge_index[b, :, 1]]
        w = edge_weight[b]
        half = dst >= HALF0
        for h in (0, 1):
            m = half == (h == 1)
            d = dst[m] - h * HALF0
            s_rows = src[m]
            sh = s_rows >= SRC_SPLIT
            sloc = (s_rows - sh * SRC_SPLIT).astype(np.int16)
            bucket = (d >> 7) * 2 + sh
            order = np.argsort(bucket, kind="stable")
            bs = bucket[order]
            counts = np.bincount(bs, minlength=NSEG)
            gh = max(gh, int(np.ceil(counts.max() / P)))
            per_core.append((sloc[order], (d & 127)[order].astype(np.float32),
                             w[m][order], bs, counts))

    ngrp = NSEG * gh
    slots = ngrp * P
    in_maps = []
    for core in range(N_CORES):
        sloc, dloc, wv, bs, counts = per_core[core]
        starts = np.zeros(NSEG, np.int64)
        starts[1:] = np.cumsum(counts)[:-1]
        rank = np.arange(len(bs)) - starts[bs]
        slot = bs.astype(np.int64) * (gh * P) + rank

        sl = np.zeros(slots, np.int16)  # pads gather row 0 with w=0
        dl = np.zeros(slots, np.float32)
        wl = np.zeros(slots, np.float32)
        sl[slot] = sloc
        dl[slot] = dloc
        wl[slot] = wv

        # idx element e of segment k -> [e % 16, k*gh*8 + e//16], replicated x8
        idx16 = sl.reshape(NSEG, gh * 8, 16).transpose(2, 0, 1).reshape(16, NSEG * gh * 8)
        idx128 = np.ascontiguousarray(np.tile(idx16, (8, 1)))
        mdst = np.ascontiguousarray(dl.reshape(ngrp, P).T)
        mw = np.ascontiguousarray(wl.reshape(ngrp, P).T)

        in_maps.append({
            "h": H[core // 2].reshape(N_NODES, C),
            "idx": idx128,
            "mdst": mdst,
            "mw": mw,
            "iota": iota,
        })

    global _last_in_maps
    _last_in_maps = in_maps
    nc = _prog_cache.get(gh)
    if nc is None:
        nc = _build_program(gh)
        _prog_cache[gh] = nc

    res = run_bass_kernel_spmd(nc, in_maps, list(range(N_CORES)))

    out = np.empty((B, N_NODES, HS, HS), np.float32)
    for b in range(B):
        r0 = res.results[2 * b]["out"]
        r1 = res.results[2 * b + 1]["out"]
        out[b, :HALF0] = r0.reshape(-1, HS, HS)
        out[b, HALF0:] = r1[:N_NODES - HALF0].reshape(-1, HS, HS)
    return out



# revision 2
# speedup vs baseline: 1.0602x; 1.0602x over previous
"""Neighbor aggregation (GNN message passing) on 8 Trainium2 cores — v5.

vs v4: edges sharing (dst-block, src) collapse into one gather slot (the
one-hot W column becomes multi-hot — W row gets several nonzeros), and W
streams move to the ACT HWDGE queue in half-side-run chunks to cut SDMA ring
contention with the gathers.
"""

import sys

sys.path.insert(0, "/opt/trn_rl_repo")

import numpy as np

import concourse.bacc as bacc
import concourse.tile as tile
from concourse import mybir
from concourse.bass_utils import run_bass_kernel_spmd

B = 4
N_NODES = 50000
HS = 16
C = HS * HS
P = 128
NBLK = 196
HALF0 = NBLK * P
GLO = 9
GPB = 2 * GLO
NGRP = NBLK * GPB         # 3528
HI_BASE = 17232
N_QUEUES = 4
SB = 8

_prog_cache: dict = {}
_last_in_maps: list | None = None


def _group_order():
    order = []
    for sb0 in range(0, NBLK, SB):
        nb = min(SB, NBLK - sb0)
        for s in (0, 1):
            for jj in range(nb):
                for k in range(GLO):
                    order.append(((sb0 + jj) * GPB + s * GLO + k))
    pos = np.empty(NGRP, np.int64)
    pos[np.asarray(order)] = np.arange(NGRP)
    return np.asarray(order), pos


GROUP_AT_POS, POS_OF_GROUP = _group_order()


def _build_program():
    nc = bacc.Bacc("TRN2", target_bir_lowering=False, debug=False,
                   num_swdge_queues=N_QUEUES)
    h_d = nc.dram_tensor("h", (50048, C), mybir.dt.float16, kind="ExternalInput")
    idx_d = nc.dram_tensor("idx", (P, NGRP * 8), mybir.dt.int16, kind="ExternalInput")
    w_d = nc.dram_tensor("w", (P, NGRP * P), mybir.dt.float16, kind="ExternalInput")
    out_d = nc.dram_tensor("out", (NBLK * P, C), mybir.dt.float32, kind="ExternalOutput")

    h_ap = h_d.ap()
    win_aps = (h_ap[0:32768, :], h_ap[HI_BASE:HI_BASE + 32768, :])
    qctr = [0]

    with tile.TileContext(nc) as tc:
        with tc.tile_pool(name="const", bufs=1) as cpool, \
             tc.tile_pool(name="gat", bufs=20) as gpool, \
             tc.tile_pool(name="gat4", bufs=4) as g4pool, \
             tc.tile_pool(name="wt", bufs=4) as wpool, \
             tc.tile_pool(name="otile", bufs=4) as opool, \
             tc.tile_pool(name="psum", bufs=8, space="PSUM") as ppool:
            idx_t = cpool.tile([P, NGRP * 8], mybir.dt.int16)
            nc.sync.dma_start(out=idx_t[:], in_=idx_d.ap())

            pos0 = 0
            for sb0 in range(0, NBLK, SB):
                nb = min(SB, NBLK - sb0)
                run = nb * GLO
                gtiles = []
                wtiles = []
                for s in (0, 1):
                    side_g = []
                    base = pos0 + s * run
                    for c0 in range(0, run, 8):
                        ng = min(8, run - c0)
                        gp = gpool if ng == 8 else g4pool
                        t = gp.tile([P, ng, C], mybir.dt.float16, tag=f"g{ng}")
                        nc.gpsimd.dma_gather(
                            out_ap=t[:],
                            in_ap=win_aps[s],
                            idxs_ap=idx_t[:, (base + c0) * 8:(base + c0 + ng) * 8],
                            num_idxs=ng * P,
                            num_idxs_reg=ng * P,
                            elem_size=C,
                            queue_num=qctr[0] % N_QUEUES,
                        )
                        qctr[0] += 1
                        side_g.append((t, base + c0, ng))
                    gtiles.append(side_g)
                    half = (run + 1) // 2
                    for w0 in (0, half):
                        nw = (half if w0 == 0 else run - half)
                        wt = wpool.tile([P, nw, P], mybir.dt.float16,
                                        tag=f"w{nw}")
                        nc.scalar.dma_start(
                            out=wt[:],
                            in_=w_d.ap()[:, (base + w0) * P:(base + w0 + nw) * P])
                        wtiles.append((wt, base + w0, nw))

                for jj in range(nb):
                    acc = ppool.tile([P, C], mybir.dt.float32, space="PSUM")
                    for kk in range(GPB):
                        s, k = divmod(kk, GLO)
                        pos = pos0 + s * run + jj * GLO + k
                        t, t0, _ = next(p for p in gtiles[s]
                                        if p[1] <= pos < p[1] + p[2])
                        wt, w0, _ = next(p for p in wtiles
                                         if p[1] <= pos < p[1] + p[2])
                        nc.tensor.matmul(
                            out=acc[:], lhsT=wt[:, pos - w0, :],
                            rhs=t[:, pos - t0, :],
                            start=(kk == 0), stop=(kk == GPB - 1))
                    j = sb0 + jj
                    ot = opool.tile([P, C], mybir.dt.float32, tag="out")
                    nc.vector.tensor_copy(out=ot[:], in_=acc[:])
                    nc.sync.dma_start(out=out_d.ap()[j * P:(j + 1) * P, :],
                                      in_=ot[:])
                pos0 += 2 * run

    nc.compile()
    return nc


def kernel(H, edge_index, edge_weight, node_idx):
    H = np.asarray(H, dtype=np.float32)
    edge_index = np.asarray(edge_index)
    edge_weight = np.ascontiguousarray(np.asarray(edge_weight), dtype=np.float32)
    node_idx = np.asarray(node_idx)

    inv = np.argsort(node_idx).astype(np.int64)
    slots_per_side = GLO * P

    in_maps = []
    for core in range(2 * B):
        b, half = divmod(core, 2)
        dst = inv[edge_index[b, :, 0]]
        src = inv[edge_index[b, :, 1]]
        w = edge_weight[b]
        m = (dst >= half * HALF0) & (dst < (half + 1) * HALF0)
        d = (dst[m] - half * HALF0).astype(np.int64)
        s = src[m]
        wv = w[m]

        blk = d >> 7
        hi = s >= 25000
        bucket = blk * 2 + hi
        pair = bucket * 50000 + s          # dedup same-(block, src) edges
        uniq, inv_e = np.unique(pair, return_inverse=True)
        ubucket = uniq // 50000
        usrc = uniq % 50000
        counts = np.bincount(ubucket, minlength=NBLK * 2)
        if counts.max() > slots_per_side:
            raise RuntimeError(f"bucket overflow: {counts.max()}")
        starts = np.zeros(NBLK * 2, np.int64)
        starts[1:] = np.cumsum(counts)[:-1]
        urank = np.arange(len(uniq)) - starts[ubucket]
        canon = (ubucket >> 1) * GPB + (ubucket & 1) * GLO + (urank // P)
        upos = POS_OF_GROUP[canon]
        uerow = urank % P

        sl = np.zeros(NGRP * P, np.int16)
        sl[upos * P + uerow] = (
            usrc - np.where((ubucket & 1) == 1, HI_BASE, 0)).astype(np.int16)
        idx16 = sl.reshape(NGRP, 8, 16).transpose(2, 0, 1).reshape(16, NGRP * 8)
        idx128 = np.ascontiguousarray(np.tile(idx16, (8, 1)))

        # host-built multi-hot W: wbig[e, pos*128 + dstloc] += w_e
        wbig32 = np.zeros((P, NGRP * P), np.float32)
        np.add.at(wbig32, (uerow[inv_e], upos[inv_e] * P + (d & 127)), wv)
        wbig = wbig32.astype(np.float16)

        h16 = np.zeros((50048, C), np.float16)
        h16[:N_NODES] = H[b].reshape(N_NODES, C).astype(np.float16)

        in_maps.append({"h": h16, "idx": idx128, "w": wbig})

    global _last_in_maps
    _last_in_maps = in_maps
    nc = _prog_cache.get("v5")
    if nc is None:
        nc = _build_program()
        _prog_cache["v5"] = nc

    res = run_bass_kernel_spmd(nc, in_maps, list(range(2 * B)))

    out = np.empty((B, N_NODES, HS, HS), np.float32)
    for b in range(B):
        r0 = res.results[2 * b]["out"]
        r1 = res.results[2 * b + 1]["out"]
        out[b, :HALF0] = r0.reshape(-1, HS, HS)
        out[b, HALF0:] = r1[:N_NODES - HALF0].reshape(-1, HS, HS)
    return out
